# revision 19
# baseline (speedup 1.0000x reference)
"""Context-parallel causal attention block on 8 Trainium2 NeuronCores.

Strategy: tensor-parallel split-heads. Each core c computes Q/K/V projections
for its 2 heads (of 16) over all tokens with host-sliced weights, runs causal
attention locally (feature-major layouts, no transposes), then re-shards from
head-parallel to token-parallel with FOUR pipelined AllToAll chunks (one per
(head-slot, batch)), and each core runs the output projection for its 512
owned tokens (256 from each batch) as two passes that overlap the last
collectives. Softmax denominators are accumulated on the (otherwise idle)
Pool engine instead of PE ones-matmuls.

Matmul operands are bf16 (fp32 matmuls are two-pass / half-rate on TRN2's PE);
all accumulation stays fp32 in PSUM, softmax runs on fp32 scores.
"""
import sys

sys.path.insert(0, "/opt/trn_rl_repo")

import ml_dtypes
import numpy as np

import concourse.bass as bass
import concourse.tile as tile
from concourse import bacc, mybir
from concourse.bass_utils import run_bass_kernel_spmd

FP = mybir.dt.float32
BF = mybir.dt.bfloat16
NPBF = ml_dtypes.bfloat16
N_CORES = 8
B, S, D, H, DH = 2, 2048, 2048, 16, 128
T = B * S            # 4096 flattened tokens, b-major
KK = D // 128        # 16 contraction k-tiles
NEG = -1.0e30


def build_nc() -> bacc.Bacc:
    nc = bacc.Bacc("TRN2", target_bir_lowering=False, debug=False, num_devices=N_CORES)

    xt = nc.dram_tensor("xt", [128, KK, T], BF, kind="ExternalInput")
    wq = nc.dram_tensor("wq", [128, KK, 256], BF, kind="ExternalInput")
    wk = nc.dram_tensor("wk", [128, KK, 256], BF, kind="ExternalInput")
    wv = nc.dram_tensor("wv", [128, KK, 256], BF, kind="ExternalInput")
    wo = nc.dram_tensor("wo", [128, KK, D], BF, kind="ExternalInput")
    # columns: [b0 tokens c*256:(c+1)*256 | b1 tokens c*256:(c+1)*256]
    out_t = nc.dram_tensor("out_t", [D, 512], FP, kind="ExternalOutput")

    with tile.TileContext(nc) as tc:
        with (
            tc.tile_pool(name="dram", bufs=1, space="DRAM") as dram,
            tc.tile_pool(name="consts", bufs=1) as consts,
            tc.tile_pool(name="persist", bufs=1) as persist,
            tc.tile_pool(name="wop", bufs=16) as wop,
        ):
            # a2a chunk (hl, b): slice j -> dest core j gets [128 feat, 256 tok]
            a2a_in = [[dram.tile([N_CORES, 128, 256], BF, name=f"a2a_in{hl}{b}")
                       for b in range(B)] for hl in range(2)]
            a2a_out = [[dram.tile([N_CORES, 128, 256], BF, name=f"a2a_out{hl}{b}")
                        for b in range(B)] for hl in range(2)]

            ones = consts.tile([128, 1], BF)
            nc.gpsimd.memset(ones[:], 1.0)
            # additive causal masks for the 4 diagonal offsets:
            # masks[p, i, q] = 0 if q >= p + i*128 else NEG
            masks = consts.tile([128, 4, 512], FP)
            nc.gpsimd.memset(masks[:], 0.0)
            for i in range(4):
                nc.gpsimd.affine_select(
                    out=masks[:, i, :],
                    in_=masks[:, i, :],
                    compare_op=mybir.AluOpType.is_ge,
                    fill=NEG,
                    base=-(i * 128),
                    pattern=[[1, 512]],
                    channel_multiplier=-1,
                )

            qT = persist.tile([128, 2, T], BF)       # [dh, hl, t]
            kT = persist.tile([128, 2, T], BF)
            v_sb = persist.tile([128, 32, 256], BF)  # [t%128, t//128, head_feat]
            wo_tiles = []

            with (
                tc.tile_pool(name="wpool", bufs=1) as wpool,
                tc.tile_pool(name="xtp", bufs=6) as xtp,
                tc.tile_pool(name="ps1", bufs=2, space="PSUM") as ps1,
                tc.tile_pool(name="expp", bufs=3) as expp,
                tc.tile_pool(name="smallp", bufs=2) as smallp,
                tc.tile_pool(name="otp", bufs=3) as otp,
                tc.tile_pool(name="otsb", bufs=2) as otsb,
                tc.tile_pool(name="otsb1", bufs=1) as otsb1,
                tc.tile_pool(name="psT", bufs=2, space="PSUM") as psT,
                tc.tile_pool(name="psA", bufs=1, space="PSUM") as psA,
                tc.tile_pool(name="psS", bufs=1, space="PSUM") as psS,
            ):
                wq_sb = wpool.tile([128, KK, 256], BF)
                wk_sb = wpool.tile([128, KK, 256], BF)
                wv_sb = wpool.tile([128, KK, 256], BF)
                def proj_strip(b, s):
                    strip = b * 4 + s
                    t0 = strip * 512
                    xq = []
                    for qtr in range(4):
                        if strip == 0:
                            # interleave weight quarters with the first x strip
                            # so Q-pass kk=0 can start after ~2 small DMAs
                            nc.sync.dma_start(
                                wq_sb[:, qtr * 4 : (qtr + 1) * 4, :],
                                wq[:, qtr * 4 : (qtr + 1) * 4, :],
                            )
                        xtile = xtp.tile([128, 4, 512], BF, tag="xt")
                        nc.sync.dma_start(
                            xtile[:],
                            xt[:, qtr * 4 : (qtr + 1) * 4, t0 : t0 + 512],
                        )
                        xq.append(xtile)
                    if strip == 0:
                        nc.sync.dma_start(wk_sb[:], wk[:])
                        nc.sync.dma_start(wv_sb[:], wv[:])

                    # pass A: q for both heads (2 banks)
                    pa = [ps1.tile([128, 512], FP, tag=f"p1{j}", name=f"pa{j}")
                          for j in range(2)]
                    for kk in range(KK):
                        xsl = xq[kk // 4][:, kk % 4, :]
                        st, sp = kk == 0, kk == KK - 1
                        nc.tensor.matmul(pa[0][:], wq_sb[:, kk, 0:128], xsl, start=st, stop=sp)
                        nc.tensor.matmul(pa[1][:], wq_sb[:, kk, 128:256], xsl, start=st, stop=sp)
                    for hl in range(2):
                        nc.scalar.copy(qT[:, hl, t0 : t0 + 512], pa[hl][:])
                    # pass B: k for both heads
                    pb = [ps1.tile([128, 512], FP, tag=f"p1{j}", name=f"pb{j}")
                          for j in range(2)]
                    for kk in range(KK):
                        xsl = xq[kk // 4][:, kk % 4, :]
                        st, sp = kk == 0, kk == KK - 1
                        nc.tensor.matmul(pb[0][:], wk_sb[:, kk, 0:128], xsl, start=st, stop=sp)
                        nc.tensor.matmul(pb[1][:], wk_sb[:, kk, 128:256], xsl, start=st, stop=sp)
                    for hl in range(2):
                        nc.scalar.copy(kT[:, hl, t0 : t0 + 512], pb[hl][:])
                    # pass C/D: v in two tt sub-passes (2 banks each)
                    for half in range(2):
                        pv = [ps1.tile([128, 256], FP, tag=f"p1{j}", name=f"pv{j}")
                              for j in range(2)]
                        for kk in range(KK):
                            xsl = xq[kk // 4][:, kk % 4, :]
                            st, sp = kk == 0, kk == KK - 1
                            for jj in range(2):
                                tt = half * 2 + jj
                                nc.tensor.matmul(
                                    pv[jj][:],
                                    xsl[:, tt * 128 : (tt + 1) * 128],
                                    wv_sb[:, kk, :],
                                    start=st,
                                    stop=sp,
                                )
                        for jj in range(2):
                            nc.vector.tensor_copy(
                                v_sb[:, strip * 4 + half * 2 + jj, :], pv[jj][:]
                            )

                def attention_unit(b, hl, s):
                    q0 = b * S + s * 512
                    qts = qT[:, hl, q0 : q0 + 512]
                    avp = psA.tile([128, 512], FP, tag="av")
                    smp = psS.tile([1, 512], FP, tag="sm")
                    nk = 4 * (s + 1)
                    if s > 0:
                        # diagonal tiles first, truncated to their causally live
                        # columns; the trailing full-width past tiles then give a
                        # clean single accumulation group (start on first, full-
                        # width; stop on last)
                        order = list(range(4 * s, nk)) + list(range(0, 4 * s))
                    else:
                        order = list(range(nk))
                    for idx, ki in enumerate(order):
                        di = ki - 4 * s
                        c0 = di * 128 if (s > 0 and di > 0) else 0
                        stp = psT.tile([128, 512], FP, tag="st")
                        nc.tensor.matmul(
                            stp[:, c0:512],
                            kT[:, hl, b * S + ki * 128 : b * S + (ki + 1) * 128],
                            qts[:, c0:512],
                            start=True,
                            stop=True,
                        )
                        if di >= 0:
                            nc.vector.tensor_add(
                                stp[:, c0:512], stp[:, c0:512], masks[:, di, c0:512]
                            )
                        ex = expp.tile([128, 512], BF, tag="ex")
                        nc.scalar.activation(
                            ex[:, c0:512], stp[:, c0:512],
                            mybir.ActivationFunctionType.Exp,
                        )
                        st, sp = idx == 0, idx == len(order) - 1
                        nc.tensor.matmul(
                            avp[:, c0:512],
                            v_sb[:, b * 16 + ki, hl * 128 : (hl + 1) * 128],
                            ex[:, c0:512],
                            start=st,
                            stop=sp,
                        )
                        nc.tensor.matmul(
                            smp[:, c0:512], ones[:], ex[:, c0:512], start=st, stop=sp
                        )
                    sums_sb = smallp.tile([1, 512], FP, tag="sums")
                    nc.scalar.copy(sums_sb[:], smp[:])
                    sbc = smallp.tile([128, 512], FP, tag="sbc")
                    nc.gpsimd.partition_broadcast(sbc[:], sums_sb[:])
                    rbc = smallp.tile([128, 512], FP, tag="rbc")
                    nc.vector.reciprocal_approx_fast(rbc[:], sbc[:])
                    ot = otp.tile([128, 512], BF, tag="ot")
                    nc.vector.tensor_mul(ot[:], avp[:], rbc[:])
                    # strip s covers dest cores 2s (cols 0:256) and 2s+1
                    nc.sync.dma_start(a2a_in[hl][b][2 * s, :, :], ot[:, 0:256])
                    nc.sync.dma_start(a2a_in[hl][b][2 * s + 1, :, :], ot[:, 256:512])

                def fire_a2a(hl, b):
                    nc.gpsimd.collective_compute(
                        "AllToAll", mybir.AluOpType.bypass,
                        replica_groups=[list(range(N_CORES))],
                        ins=[a2a_in[hl][b][:].opt()],
                        outs=[a2a_out[hl][b][:].opt()],
                    )

                def outproj_gather(hl, b):
                    # issued right after chunk (hl, b) fires: each half-gather
                    # depends on exactly one collective, so the sync-queue wait
                    # resolves as early as possible.  hl=1 gathers share ONE
                    # buffer: the WAR hazard pins gather(1,1) behind outproj(0)'s
                    # reads of gather(1,0), so the scheduler cannot hoist it (and
                    # its cc>=4 semaphore count) ahead of outproj(0).
                    pool = otsb if hl == 0 else otsb1
                    g = pool.tile([128, 8, 256], BF, tag=f"otsb{hl}", name=f"otsb{hl}{b}")
                    for i in range(N_CORES):
                        nc.sync.dma_start(g[:, i, :], a2a_out[hl][b][i, :, :])
                    return g

                def outproj(b, g0, g1):
                    # g0[:, i, :] = head 2i, g1[:, i, :] = head 2i+1; accumulate
                    # even slots first (their gather lands earliest)
                    for dd in range(KK):
                        op = ps1.tile([128, 512], FP, tag="p10", name=f"op{b}_{dd}")
                        for j in range(16):
                            g, i = (g0, j) if j < 8 else (g1, j - 8)
                            nc.tensor.matmul(
                                op[:, 0:256],
                                wo_tiles[dd][:, 2 * i + (j >= 8), :],
                                g[:, i, :],
                                start=(j == 0),
                                stop=(j == 15),
                            )
                        ob = otp.tile([128, 256], FP, tag="ob")
                        nc.scalar.copy(ob[:], op[:, 0:256])
                        nc.sync.dma_start(
                            out_t[dd * 128 : (dd + 1) * 128, b * 256 : (b + 1) * 256],
                            ob[:],
                        )

                gth = {}
                for b in range(B):
                    for s in range(4):
                        proj_strip(b, s)
                        attention_unit(b, 0, s)
                    if b == 1:
                        # placed before fire(0,1): precise cc>=2 wait, and the
                        # sync queue reaches here only after b1's x DMAs issued
                        gth[(1, 0)] = outproj_gather(1, 0)
                    fire_a2a(0, b)
                    gth[(0, b)] = outproj_gather(0, b)
                    if b == 0:
                        # prefetch output-projection weights during h1(b0)
                        for dd in range(KK):
                            wod = wop.tile([128, KK, 128], BF, tag="wod", name=f"wod{dd}")
                            nc.sync.dma_start(wod[:], wo[:, :, dd * 128 : (dd + 1) * 128])
                            wo_tiles.append(wod)
                    for s in range(4):
                        attention_unit(b, 1, s)
                    fire_a2a(1, b)

                outproj(0, gth[(0, 0)], gth[(1, 0)])  # overlaps the (1,1) AllToAll
                # gather(1,1) issued only now: any earlier and outproj(0)'s
                # instructions inherit its cc>=4 wait through shared semaphores
                gth[(1, 1)] = outproj_gather(1, 1)
                outproj(1, gth[(0, 1)], gth[(1, 1)])

    nc.compile()
    return nc


_NC_CACHE = {}


def _get_nc():
    if "nc" not in _NC_CACHE:
        _NC_CACHE["nc"] = build_nc()
    return _NC_CACHE["nc"]


def _make_in_maps(x, wq, wk, wv, wo):
    x = np.ascontiguousarray(np.asarray(x, dtype=np.float32))
    wq = np.asarray(wq, dtype=np.float32)
    wk = np.asarray(wk, dtype=np.float32)
    wv = np.asarray(wv, dtype=np.float32)
    wo = np.asarray(wo, dtype=np.float32)

    x_flat = x.reshape(T, D)
    # xt[p, kk, t] = x_flat[t, kk*128+p]
    xt_host = np.ascontiguousarray(
        x_flat.T.reshape(KK, 128, T).transpose(1, 0, 2)
    ).astype(NPBF)
    # wo_dev[p, ff, d] = wo[d, ff*128+p]
    wo_host = np.ascontiguousarray(
        wo.T.reshape(KK, 128, D).transpose(1, 0, 2)
    ).astype(NPBF)
    scale = 1.0 / np.sqrt(np.float32(DH))

    in_maps = []
    for c in range(N_CORES):
        sl = slice(c * 256, (c + 1) * 256)

        def wslice(w, scaled=False):
            wc = w[sl, :].T  # [D, 256]
            if scaled:
                wc = wc * scale
            return np.ascontiguousarray(
                wc.reshape(KK, 128, 256).transpose(1, 0, 2)
            ).astype(NPBF)

        in_maps.append(
            {
                "xt": xt_host,
                "wq": wslice(wq, scaled=True),
                "wk": wslice(wk),
                "wv": wslice(wv),
                "wo": wo_host,
            }
        )
    return in_maps


def _run(x, wq, wk, wv, wo, trace=False):
    nc = _get_nc()
    in_maps = _make_in_maps(x, wq, wk, wv, wo)
    res = run_bass_kernel_spmd(nc, in_maps, list(range(N_CORES)), trace=trace)
    out = np.empty((B, S, D), dtype=np.float32)
    for c in range(N_CORES):
        o = res.results[c]["out_t"]  # [D, 512]
        out[0, c * 256 : (c + 1) * 256, :] = o[:, 0:256].T
        out[1, c * 256 : (c + 1) * 256, :] = o[:, 256:512].T
    return out, res


def kernel(x, wq, wk, wv, wo):
    out, _ = _run(x, wq, wk, wv, wo, trace=False)
    return out


# revision 20
# speedup vs baseline: 1.0038x; 1.0038x over previous
"""Context-parallel causal attention block on 8 Trainium2 NeuronCores.

Strategy: tensor-parallel split-heads. Each core c computes Q/K/V projections
for its 2 heads (of 16) over all tokens with host-sliced weights, runs causal
attention locally (feature-major layouts, no transposes), then re-shards from
head-parallel to token-parallel with FOUR pipelined AllToAll chunks (one per
(head-slot, batch)), and each core runs the output projection for its 512
owned tokens (256 from each batch) as two passes that overlap the last
collectives. Softmax denominators are accumulated on the (otherwise idle)
Pool engine instead of PE ones-matmuls.

Matmul operands are bf16 (fp32 matmuls are two-pass / half-rate on TRN2's PE);
all accumulation stays fp32 in PSUM, softmax runs on fp32 scores.
"""
import sys

sys.path.insert(0, "/opt/trn_rl_repo")

import ml_dtypes
import numpy as np

import concourse.bass as bass
import concourse.tile as tile
from concourse import bacc, mybir
from concourse.bass_utils import run_bass_kernel_spmd

FP = mybir.dt.float32
BF = mybir.dt.bfloat16
NPBF = ml_dtypes.bfloat16
N_CORES = 8
B, S, D, H, DH = 2, 2048, 2048, 16, 128
T = B * S            # 4096 flattened tokens, b-major
KK = D // 128        # 16 contraction k-tiles
NEG = -1.0e30


def build_nc() -> bacc.Bacc:
    nc = bacc.Bacc("TRN2", target_bir_lowering=False, debug=False, num_devices=N_CORES)

    xt = nc.dram_tensor("xt", [128, KK, T], BF, kind="ExternalInput")
    wq = nc.dram_tensor("wq", [128, KK, 256], BF, kind="ExternalInput")
    wk = nc.dram_tensor("wk", [128, KK, 256], BF, kind="ExternalInput")
    wv = nc.dram_tensor("wv", [128, KK, 256], BF, kind="ExternalInput")
    wo = nc.dram_tensor("wo", [128, KK, D], BF, kind="ExternalInput")
    # columns: [b0 tokens c*256:(c+1)*256 | b1 tokens c*256:(c+1)*256]
    out_t = nc.dram_tensor("out_t", [D, 512], FP, kind="ExternalOutput")

    with tile.TileContext(nc) as tc:
        with (
            tc.tile_pool(name="dram", bufs=1, space="DRAM") as dram,
            tc.tile_pool(name="consts", bufs=1) as consts,
            tc.tile_pool(name="persist", bufs=1) as persist,
            tc.tile_pool(name="wop", bufs=16) as wop,
        ):
            # a2a chunk (hl, b): slice j -> dest core j gets [128 feat, 256 tok]
            a2a_in = [[dram.tile([N_CORES, 128, 256], BF, name=f"a2a_in{hl}{b}")
                       for b in range(B)] for hl in range(2)]
            a2a_out = [[dram.tile([N_CORES, 128, 256], BF, name=f"a2a_out{hl}{b}")
                        for b in range(B)] for hl in range(2)]

            ones = consts.tile([128, 1], BF)
            nc.gpsimd.memset(ones[:], 1.0)
            # additive causal masks for the 4 diagonal offsets:
            # masks[p, i, q] = 0 if q >= p + i*128 else NEG
            masks = consts.tile([128, 4, 512], FP)
            nc.gpsimd.memset(masks[:], 0.0)
            for i in range(4):
                nc.gpsimd.affine_select(
                    out=masks[:, i, :],
                    in_=masks[:, i, :],
                    compare_op=mybir.AluOpType.is_ge,
                    fill=NEG,
                    base=-(i * 128),
                    pattern=[[1, 512]],
                    channel_multiplier=-1,
                )

            qT = persist.tile([128, 2, T], BF)       # [dh, hl, t]
            kT = persist.tile([128, 2, T], BF)
            v_sb = persist.tile([128, 32, 256], BF)  # [t%128, t//128, head_feat]
            wo_tiles = []

            with (
                tc.tile_pool(name="wpool", bufs=1) as wpool,
                tc.tile_pool(name="xtp", bufs=6) as xtp,
                tc.tile_pool(name="ps1", bufs=2, space="PSUM") as ps1,
                tc.tile_pool(name="expp", bufs=3) as expp,
                tc.tile_pool(name="smallp", bufs=2) as smallp,
                tc.tile_pool(name="otp", bufs=3) as otp,
                tc.tile_pool(name="otsb", bufs=2) as otsb,
                tc.tile_pool(name="otsb1", bufs=1) as otsb1,
                tc.tile_pool(name="psT", bufs=2, space="PSUM") as psT,
                tc.tile_pool(name="psA", bufs=1, space="PSUM") as psA,
                tc.tile_pool(name="psS", bufs=1, space="PSUM") as psS,
            ):
                wq_sb = wpool.tile([128, KK, 256], BF)
                wk_sb = wpool.tile([128, KK, 256], BF)
                wv_sb = wpool.tile([128, KK, 256], BF)
                def proj_strip(b, s):
                    strip = b * 4 + s
                    t0 = strip * 512
                    xq = []
                    for qtr in range(4):
                        if strip == 0:
                            # interleave weight quarters with the first x strip
                            # so Q-pass kk=0 can start after ~2 small DMAs
                            nc.sync.dma_start(
                                wq_sb[:, qtr * 4 : (qtr + 1) * 4, :],
                                wq[:, qtr * 4 : (qtr + 1) * 4, :],
                            )
                        xtile = xtp.tile([128, 4, 512], BF, tag="xt")
                        nc.sync.dma_start(
                            xtile[:],
                            xt[:, qtr * 4 : (qtr + 1) * 4, t0 : t0 + 512],
                        )
                        xq.append(xtile)
                    if strip == 0:
                        nc.sync.dma_start(wk_sb[:], wk[:])
                        nc.sync.dma_start(wv_sb[:], wv[:])

                    # pass A: q for both heads (2 banks)
                    pa = [ps1.tile([128, 512], FP, tag=f"p1{j}", name=f"pa{j}")
                          for j in range(2)]
                    for kk in range(KK):
                        xsl = xq[kk // 4][:, kk % 4, :]
                        st, sp = kk == 0, kk == KK - 1
                        nc.tensor.matmul(pa[0][:], wq_sb[:, kk, 0:128], xsl, start=st, stop=sp)
                        nc.tensor.matmul(pa[1][:], wq_sb[:, kk, 128:256], xsl, start=st, stop=sp)
                    for hl in range(2):
                        nc.scalar.copy(qT[:, hl, t0 : t0 + 512], pa[hl][:])
                    # pass B: k for both heads
                    pb = [ps1.tile([128, 512], FP, tag=f"p1{j}", name=f"pb{j}")
                          for j in range(2)]
                    for kk in range(KK):
                        xsl = xq[kk // 4][:, kk % 4, :]
                        st, sp = kk == 0, kk == KK - 1
                        nc.tensor.matmul(pb[0][:], wk_sb[:, kk, 0:128], xsl, start=st, stop=sp)
                        nc.tensor.matmul(pb[1][:], wk_sb[:, kk, 128:256], xsl, start=st, stop=sp)
                    for hl in range(2):
                        nc.scalar.copy(kT[:, hl, t0 : t0 + 512], pb[hl][:])
                    # pass C/D: v in two tt sub-passes (2 banks each)
                    for half in range(2):
                        pv = [ps1.tile([128, 256], FP, tag=f"p1{j}", name=f"pv{j}")
                              for j in range(2)]
                        for kk in range(KK):
                            xsl = xq[kk // 4][:, kk % 4, :]
                            st, sp = kk == 0, kk == KK - 1
                            for jj in range(2):
                                tt = half * 2 + jj
                                nc.tensor.matmul(
                                    pv[jj][:],
                                    xsl[:, tt * 128 : (tt + 1) * 128],
                                    wv_sb[:, kk, :],
                                    start=st,
                                    stop=sp,
                                )
                        for jj in range(2):
                            nc.vector.tensor_copy(
                                v_sb[:, strip * 4 + half * 2 + jj, :], pv[jj][:]
                            )

                def attention_unit(b, hl, s):
                    q0 = b * S + s * 512
                    qts = qT[:, hl, q0 : q0 + 512]
                    avp = psA.tile([128, 512], FP, tag="av")
                    smp = psS.tile([1, 512], FP, tag="sm")
                    nk = 4 * (s + 1)
                    if s > 0:
                        # diagonal tiles first, truncated to their causally live
                        # columns; the trailing full-width past tiles then give a
                        # clean single accumulation group (start on first, full-
                        # width; stop on last)
                        order = list(range(4 * s, nk)) + list(range(0, 4 * s))
                    else:
                        order = list(range(nk))
                    for idx, ki in enumerate(order):
                        di = ki - 4 * s
                        c0 = di * 128 if (s > 0 and di > 0) else 0
                        stp = psT.tile([128, 512], FP, tag="st")
                        nc.tensor.matmul(
                            stp[:, c0:512],
                            kT[:, hl, b * S + ki * 128 : b * S + (ki + 1) * 128],
                            qts[:, c0:512],
                            start=True,
                            stop=True,
                        )
                        if di >= 0:
                            nc.vector.tensor_add(
                                stp[:, c0:512], stp[:, c0:512], masks[:, di, c0:512]
                            )
                        ex = expp.tile([128, 512], BF, tag="ex")
                        nc.scalar.activation(
                            ex[:, c0:512], stp[:, c0:512],
                            mybir.ActivationFunctionType.Exp,
                        )
                        st, sp = idx == 0, idx == len(order) - 1
                        nc.tensor.matmul(
                            avp[:, c0:512],
                            v_sb[:, b * 16 + ki, hl * 128 : (hl + 1) * 128],
                            ex[:, c0:512],
                            start=st,
                            stop=sp,
                        )
                        nc.tensor.matmul(
                            smp[:, c0:512], ones[:], ex[:, c0:512], start=st, stop=sp
                        )
                    sums_sb = smallp.tile([1, 512], FP, tag="sums")
                    nc.scalar.copy(sums_sb[:], smp[:])
                    sbc = smallp.tile([128, 512], FP, tag="sbc")
                    nc.gpsimd.partition_broadcast(sbc[:], sums_sb[:])
                    rbc = smallp.tile([128, 512], FP, tag="rbc")
                    nc.vector.reciprocal_approx_fast(rbc[:], sbc[:])
                    ot = otp.tile([128, 512], BF, tag="ot")
                    nc.vector.tensor_mul(ot[:], avp[:], rbc[:])
                    # strip s covers dest cores 2s (cols 0:256) and 2s+1
                    nc.sync.dma_start(a2a_in[hl][b][2 * s, :, :], ot[:, 0:256])
                    nc.sync.dma_start(a2a_in[hl][b][2 * s + 1, :, :], ot[:, 256:512])

                def fire_a2a(hl, b):
                    nc.gpsimd.collective_compute(
                        "AllToAll", mybir.AluOpType.bypass,
                        replica_groups=[list(range(N_CORES))],
                        ins=[a2a_in[hl][b][:].opt()],
                        outs=[a2a_out[hl][b][:].opt()],
                    )

                def outproj_gather(hl, b):
                    # issued right after chunk (hl, b) fires: each half-gather
                    # depends on exactly one collective, so the sync-queue wait
                    # resolves as early as possible.  hl=1 gathers share ONE
                    # buffer: the WAR hazard pins gather(1,1) behind outproj(0)'s
                    # reads of gather(1,0), so the scheduler cannot hoist it (and
                    # its cc>=4 semaphore count) ahead of outproj(0).
                    pool = otsb if hl == 0 else otsb1
                    g = pool.tile([128, 8, 256], BF, tag=f"otsb{hl}", name=f"otsb{hl}{b}")
                    for i in range(N_CORES):
                        nc.sync.dma_start(g[:, i, :], a2a_out[hl][b][i, :, :])
                    return g

                def outproj(b, g0, g1):
                    # g0[:, i, :] = head 2i, g1[:, i, :] = head 2i+1; accumulate
                    # even slots first (their gather lands earliest)
                    for dd in range(KK):
                        op = ps1.tile([128, 512], FP, tag="p10", name=f"op{b}_{dd}")
                        for j in range(16):
                            g, i = (g0, j) if j < 8 else (g1, j - 8)
                            nc.tensor.matmul(
                                op[:, 0:256],
                                wo_tiles[dd][:, 2 * i + (j >= 8), :],
                                g[:, i, :],
                                start=(j == 0),
                                stop=(j == 15),
                            )
                        ob = otp.tile([128, 256], FP, tag="ob")
                        nc.scalar.copy(ob[:], op[:, 0:256])
                        nc.sync.dma_start(
                            out_t[dd * 128 : (dd + 1) * 128, b * 256 : (b + 1) * 256],
                            ob[:],
                        )

                gth = {}
                for b in range(B):
                    for s in range(4):
                        proj_strip(b, s)
                        attention_unit(b, 0, s)
                    if b == 1:
                        # placed before fire(0,1): precise cc>=2 wait, and the
                        # sync queue reaches here only after b1's x DMAs issued
                        gth[(1, 0)] = outproj_gather(1, 0)
                    fire_a2a(0, b)
                    gth[(0, b)] = outproj_gather(0, b)
                    if b == 0:
                        # prefetch output-projection weights during h1(b0)
                        for dd in range(KK):
                            wod = wop.tile([128, KK, 128], BF, tag="wod", name=f"wod{dd}")
                            nc.sync.dma_start(wod[:], wo[:, :, dd * 128 : (dd + 1) * 128])
                            wo_tiles.append(wod)
                    for s in range(4):
                        attention_unit(b, 1, s)
                    if b == 0:
                        fire_a2a(1, 0)

                # outproj(0) BEFORE fire(1,1) in program order: every engine
                # queue then orders it ahead of anything chunk-4 related, so no
                # conservative semaphore count can chain it to the last
                # collective.  The fire itself is a gpsimd instruction whose
                # inputs (unit ot DMAs) are all issued above, so the collective
                # still triggers immediately and runs under outproj(0).
                outproj(0, gth[(0, 0)], gth[(1, 0)])
                fire_a2a(1, 1)
                gth[(1, 1)] = outproj_gather(1, 1)
                outproj(1, gth[(0, 1)], gth[(1, 1)])

    nc.compile()
    return nc


_NC_CACHE = {}


def _get_nc():
    if "nc" not in _NC_CACHE:
        _NC_CACHE["nc"] = build_nc()
    return _NC_CACHE["nc"]


def _make_in_maps(x, wq, wk, wv, wo):
    x = np.ascontiguousarray(np.asarray(x, dtype=np.float32))
    wq = np.asarray(wq, dtype=np.float32)
    wk = np.asarray(wk, dtype=np.float32)
    wv = np.asarray(wv, dtype=np.float32)
    wo = np.asarray(wo, dtype=np.float32)

    x_flat = x.reshape(T, D)
    # xt[p, kk, t] = x_flat[t, kk*128+p]
    xt_host = np.ascontiguousarray(
        x_flat.T.reshape(KK, 128, T).transpose(1, 0, 2)
    ).astype(NPBF)
    # wo_dev[p, ff, d] = wo[d, ff*128+p]
    wo_host = np.ascontiguousarray(
        wo.T.reshape(KK, 128, D).transpose(1, 0, 2)
    ).astype(NPBF)
    scale = 1.0 / np.sqrt(np.float32(DH))

    in_maps = []
    for c in range(N_CORES):
        sl = slice(c * 256, (c + 1) * 256)

        def wslice(w, scaled=False):
            wc = w[sl, :].T  # [D, 256]
            if scaled:
                wc = wc * scale
            return np.ascontiguousarray(
                wc.reshape(KK, 128, 256).transpose(1, 0, 2)
            ).astype(NPBF)

        in_maps.append(
            {
                "xt": xt_host,
                "wq": wslice(wq, scaled=True),
                "wk": wslice(wk),
                "wv": wslice(wv),
                "wo": wo_host,
            }
        )
    return in_maps


def _run(x, wq, wk, wv, wo, trace=False):
    nc = _get_nc()
    in_maps = _make_in_maps(x, wq, wk, wv, wo)
    res = run_bass_kernel_spmd(nc, in_maps, list(range(N_CORES)), trace=trace)
    out = np.empty((B, S, D), dtype=np.float32)
    for c in range(N_CORES):
        o = res.results[c]["out_t"]  # [D, 512]
        out[0, c * 256 : (c + 1) * 256, :] = o[:, 0:256].T
        out[1, c * 256 : (c + 1) * 256, :] = o[:, 256:512].T
    return out, res


def kernel(x, wq, wk, wv, wo):
    out, _ = _run(x, wq, wk, wv, wo, trace=False)
    return out
